# revision 56
# baseline (speedup 1.0000x reference)
"""
Multi-head attention + residual + LayerNorm Trainium2 kernel (8 NeuronCores).

Problem (hardcoded shapes):
    hidden_states [2, 2048, 1024] f32, mask [2, 2048, 2048] int32,
    Wq/Wk/Wv/Wd [1024, 1024] f32, bd/gamma/beta [1024] f32.
    out = LayerNorm(ctx @ Wd.T + bd + hidden_states) with 16 heads, hd=64.

Sharding: pure data parallel. Core c handles batch b = c//4 and query rows
q in [ (c%4)*512, (c%4)*512+512 ).  Each core computes K/V for the full
sequence of its batch (4x redundant), attention + dense + LN for its own
512 rows.  No collectives.

Key engine-balance ideas vs the naive schedule (366us -> 274us):
  * Projections (Q/K/V) and dense run as fp8e4m3 DoubleRow matmuls (2 fp8
    weights per PE cell, ~1.44x measured).  Weights are scaled x64 on host
    so their ~0.02 magnitudes sit in fp8's normal range; the scale is
    folded into the softmax exp scale (q,k), the V ones-row value (v) and
    a scale-invariant LayerNorm with eps*64^2 (dense+residual).
  * Projection units are emitted interleaved into the attention stream on
    a need-by-step basis plus an even spreading quota, so the PE runs at
    98-100% occupancy through the body.  This matters beyond throughput:
    the HAM clock gate halves the PE clock (k=4/8) after PE idle dips, so
    ANY mid-stream instruction that waits on a cross-engine chain (e.g. a
    normalization matmul gated on a DVE reciprocal, or a DVE multiply
    gated on a DMA round-trip) costs ~60-70us via clock flapping.  Hence
    softmax-sum normalization is strictly end-gated: its inputs complete
    only with the last head, so the Tile scheduler cannot hoist it.
  * Dense accumulates in PSUM with the (x64, fp16) residual added via an
    identity-matmul accumulate; LayerNorm (bn_stats / tensor_scalar)
    reads straight from PSUM, so the tail needs no DVE drain pass.
  * Startup: x arrives as fp8 chunk-contiguous (2KB lines) over the
    sync/scalar HW DGE queues plus the gpsimd SW queue; weight strips
    arrive per-head-pair so the first matmul fires after ~1MB of traffic.
"""

import os
import sys
from collections import deque
from contextlib import ExitStack

import numpy as np

for _p in ("/opt/trn_rl_repo",):
    if os.path.isdir(_p) and _p not in sys.path:
        sys.path.insert(0, _p)

import ml_dtypes  # noqa: E402

import concourse.bass as bass  # noqa: E402
import concourse.tile as tile  # noqa: E402
from concourse import bacc, mybir  # noqa: E402
from concourse.bass_utils import run_bass_kernel_spmd  # noqa: E402

BF16 = mybir.dt.bfloat16
F32 = mybir.dt.float32
FP8 = mybir.dt.float8e4
NP_BF16 = ml_dtypes.bfloat16
NP_FP8 = ml_dtypes.float8_e4m3

B, S, H, NH = 2, 2048, 1024, 16
HD = H // NH  # 64
P = 128
NCORES = 8
SQ = S // 4  # 512 query rows per core
FC = H // P  # 8 feature chunks
KC = S // P  # 16 kv chunks
NHP = FC  # 8 head pairs
WS = 64.0  # host-side weight scale (power of 2, keeps fp8 normal)
SCALE = 1.0 / (float(np.sqrt(HD)) * WS * WS)  # exp scale absorbs q,k scales
EPS = 1e-6 * WS * WS  # pre-LN tensor is x64 -> var x64^2; LN is scale-inv
CG = 2  # kv chunks per attention step
NSTEP = KC // CG  # 8 steps per head

DR = mybir.MatmulPerfMode.DoubleRow

# Results of the last device run (for test harness introspection)
last_results = None


def _build_program(affine=True):
    nc = bacc.Bacc(
        "TRN2",
        target_bir_lowering=False,
        debug=False,
        enable_asserts=False,
        num_devices=NCORES,
    )

    # Per-core DRAM inputs.  Weights are fp8, pre-scaled by WS, packed for
    # strip-wise (per head-pair / per v-half) DMA with fat lines.
    d_xT = nc.dram_tensor("xT", [FC, P, S], FP8, kind="ExternalInput").ap()
    d_wq = nc.dram_tensor("wqT", [NHP, P, FC * P], FP8, kind="ExternalInput").ap()
    d_wk = nc.dram_tensor("wkT", [NHP, P, FC * P], FP8, kind="ExternalInput").ap()
    d_wv = nc.dram_tensor("wvT", [2, P, FC * 512], FP8, kind="ExternalInput").ap()
    d_wd = nc.dram_tensor("wdT", [FC, P, H], FP8, kind="ExternalInput").ap()
    d_maskT = nc.dram_tensor("maskT", [KC, P, SQ], BF16, kind="ExternalInput").ap()
    d_xres = nc.dram_tensor("xres", [SQ // P, P, H], mybir.dt.float16,
                            kind="ExternalInput").ap()
    d_gamma = nc.dram_tensor("gamma", [H], F32, kind="ExternalInput").ap()
    d_beta = nc.dram_tensor("beta", [H], F32, kind="ExternalInput").ap()
    d_sel = nc.dram_tensor("sel", [NH, FC, P], BF16, kind="ExternalInput").ap()
    d_eye = nc.dram_tensor("eye", [P, P], mybir.dt.float16,
                           kind="ExternalInput").ap()
    d_out = nc.dram_tensor("out", [SQ // P, P, H], F32, kind="ExternalOutput").ap()

    with tile.TileContext(nc, trace_sim=False) as tc:
        _program(tc, d_xT, d_wq, d_wk, d_wv, d_wd, d_maskT, d_xres, d_gamma,
                 d_beta, d_sel, d_eye, d_out, affine)

    nc.compile()
    return nc


def _bcast_ap(src_1d, parts):
    """AP that replicates a [n] DRAM vector across `parts` partitions."""
    return bass.AP(
        tensor=src_1d.tensor,
        offset=src_1d.offset,
        ap=[[0, parts]] + list(src_1d.ap),
    )


def _program(ctx_or_tc, *args):
    with ExitStack() as ctx:
        _program_inner(ctx, ctx_or_tc, *args)


def _program_inner(ctx, tc, d_xT, d_wq, d_wk, d_wv, d_wd, d_maskT, d_xres,
                   d_gamma, d_beta, d_sel, d_eye, d_out, affine):
    nc = tc.nc

    # ---------------- pools ----------------
    persist = ctx.enter_context(tc.tile_pool(name="persist", bufs=1))
    ps_mm = ctx.enter_context(tc.tile_pool(name="ps_mm", bufs=2, space="PSUM"))
    ps_s = ctx.enter_context(tc.tile_pool(name="ps_s", bufs=2, space="PSUM"))
    ps_c = ctx.enter_context(tc.tile_pool(name="ps_c", bufs=2, space="PSUM"))

    # ---------------- persistent tiles ----------------
    xT_sb = persist.tile([P, FC, S], FP8, name="xT_sb")
    wq_sb = persist.tile([P, NHP, FC, P], FP8, name="wq_sb")
    wk_sb = persist.tile([P, NHP, FC, P], FP8, name="wk_sb")
    wv_sb = persist.tile([P, 2, FC, 512], FP8, name="wv_sb")
    wd_sb = persist.tile([P, FC, H], FP8, name="wd_sb")
    kT_hp = [persist.tile([P, S], BF16, name=f"kT{hp}") for hp in range(NHP)]
    qT_hp = [persist.tile([P, SQ], BF16, name=f"qT{hp}") for hp in range(NHP)]
    v_g = [persist.tile([P, KC, 8, HD + 1], BF16, name=f"v{g}") for g in range(2)]
    ctxT_sb = persist.tile([P, FC, SQ], BF16, name="ctxT_sb")
    ctxT_f8 = persist.tile([P, FC, SQ], FP8, name="ctxT_f8")
    maskT_sb = persist.tile([P, KC, SQ], BF16, name="maskT_sb")
    sums16 = persist.tile([NH, SQ], F32, name="sums16")
    rec_all = persist.tile([NH, SQ], BF16, name="rec_all")
    rec_f32 = persist.tile([NH, SQ], F32, name="rec_f32")
    sel_sb = persist.tile([NH, FC, P], BF16, name="sel_sb")
    eye_sb = persist.tile([P, P], mybir.dt.float16, name="eye_sb")
    xres_sb = persist.tile([P, SQ // P, H], mybir.dt.float16, name="xres_sb")

    # ---------------- DMA prefix (critical-path ordered) ----------------
    # 4 HW DGE queues: sync, scalar, vector, tensor; SW queue: gpsimd.
    # First-needed: x chunks (all 8) + wq strip0 + wk strip0 + wv half0 +
    # mask chunks 0..1.
    for g in range(2):
        nc.gpsimd.memset(v_g[g][:, :, :, HD : HD + 1], WS)
    nc.scalar.dma_start(out=eye_sb, in_=d_eye)
    nc.scalar.dma_start(out=wq_sb[:, 0], in_=d_wq[0].rearrange("p (c n) -> p c n", n=P))
    nc.sync.dma_start(out=wk_sb[:, 0], in_=d_wk[0].rearrange("p (c n) -> p c n", n=P))
    qs = [nc.sync, nc.scalar, nc.sync, nc.scalar, nc.sync, nc.scalar,
          nc.gpsimd, nc.gpsimd]
    for c in range(FC):
        qs[c].dma_start(out=xT_sb[:, c, :], in_=d_xT[c])
    nc.scalar.dma_start(out=wv_sb[:, 0], in_=d_wv[0].rearrange("p (c n) -> p c n", n=512))
    nc.scalar.dma_start(out=maskT_sb[:, 0:4, :],
                        in_=d_maskT[0:4].rearrange("c p n -> p c n"))
    # second wave (needed within the first two head-pairs)
    nc.scalar.dma_start(out=wq_sb[:, 1], in_=d_wq[1].rearrange("p (c n) -> p c n", n=P))
    nc.sync.dma_start(out=wk_sb[:, 1], in_=d_wk[1].rearrange("p (c n) -> p c n", n=P))
    nc.sync.dma_start(out=maskT_sb[:, 4:10, :],
                      in_=d_maskT[4:10].rearrange("c p n -> p c n"))
    nc.scalar.dma_start(out=maskT_sb[:, 10:16, :],
                        in_=d_maskT[10:16].rearrange("c p n -> p c n"))
    nc.sync.dma_start(out=wv_sb[:, 1], in_=d_wv[1].rearrange("p (c n) -> p c n", n=512))
    for hp in range(2, NHP):
        nc.scalar.dma_start(out=wq_sb[:, hp],
                            in_=d_wq[hp].rearrange("p (c n) -> p c n", n=P))
        nc.sync.dma_start(out=wk_sb[:, hp],
                          in_=d_wk[hp].rearrange("p (c n) -> p c n", n=P))
    nc.sync.dma_start(out=sel_sb, in_=d_sel)
    nc.gpsimd.dma_start(out=xres_sb, in_=d_xres.rearrange("r p n -> p r n"))
    nc.sync.dma_start(out=wd_sb, in_=d_wd.rearrange("c p n -> p c n"))

    # PE p-state warmup on early-arriving data (eye 32KB + wq strip 0):
    # ramps the clock before the first projection unit.
    for i in range(5):
        wps = ps_mm.tile([P, SQ], F32, name="qps", tag="mm")
        nc.tensor.matmul(wps, lhsT=eye_sb,
                         rhs=wq_sb[:, 0, :, :].rearrange("p c n -> p (c n)")[:, 0:SQ],
                         start=True, stop=True)

    work = ctx.enter_context(tc.tile_pool(name="work", bufs=3))

    # ---------------- projection units (fp8 DoubleRow) ----------------
    def unit_q(hp):
        qps = ps_mm.tile([P, SQ], F32, name="qps", tag="mm")
        for c2 in range(FC // 2):
            nc.tensor.matmul(qps,
                             lhsT=wq_sb[:, hp, 2 * c2 : 2 * c2 + 2, :],
                             rhs=xT_sb[:, 2 * c2 : 2 * c2 + 2, 0:SQ],
                             start=(c2 == 0), stop=(c2 == FC // 2 - 1),
                             perf_mode=DR)
        nc.vector.tensor_copy(qT_hp[hp], qps)

    def unit_k(hp, n):
        kps = ps_mm.tile([P, 512], F32, name="kps", tag="mm")
        for c2 in range(FC // 2):
            nc.tensor.matmul(kps,
                             lhsT=wk_sb[:, hp, 2 * c2 : 2 * c2 + 2, :],
                             rhs=xT_sb[:, 2 * c2 : 2 * c2 + 2, n * 512 : (n + 1) * 512],
                             start=(c2 == 0), stop=(c2 == FC // 2 - 1),
                             perf_mode=DR)
        nc.vector.tensor_copy(kT_hp[hp][:, n * 512 : (n + 1) * 512], kps)

    def unit_v(g, t):
        vps = ps_mm.tile([P, 512], F32, name="vps", tag="mm")
        for c2 in range(FC // 2):
            nc.tensor.matmul(vps,
                             lhsT=xT_sb[:, 2 * c2 : 2 * c2 + 2, t * P : (t + 1) * P],
                             rhs=wv_sb[:, g, 2 * c2 : 2 * c2 + 2, :],
                             start=(c2 == 0), stop=(c2 == FC // 2 - 1),
                             perf_mode=DR)
        nc.vector.tensor_copy(v_g[g][:, t, :, 0:HD],
                              vps.rearrange("p (h d) -> p h d", d=HD))

    # ---------------- unit schedule ----------------
    # Prefix: minimum to unblock head 0 step 0.
    unit_q(0)
    unit_k(0, 0)
    unit_v(0, 0)
    unit_v(0, 1)

    done_q = {0}
    done_k = {(0, 0)}
    done_v = {(0, 0), (0, 1)}

    # Remaining units ordered by earliest need-by (head, step); v-group 1
    # spread through the hp1..3 region, k strips just-in-time.
    units = deque()
    for t in range(2, KC):  # v(0,*) needed across head 0's steps
        units.append(("v", 0, t))
        if t % 4 == 3 and t // 4 < 4 and (0, t // 4) not in done_k:
            units.append(("k", 0, t // 4))
    for n in range(1, 4):
        if ("k", 0, n) not in units:
            units.append(("k", 0, n))
    for hp in range(1, NHP):
        units.append(("q", hp, 0))
        for n in range(4):
            units.append(("k", hp, n))
        if hp in (1, 2, 3):  # v group 1: 16 units over 3 head pairs
            base = (hp - 1) * 5
            for t in range(base, min(base + 5, KC)):
                units.append(("v", 1, t))
    units.append(("v", 1, 15))

    def emit_unit():
        kind, a, b2 = units.popleft()
        if kind == "q":
            unit_q(a)
            done_q.add(a)
        elif kind == "k":
            unit_k(a, b2)
            done_k.add((a, b2))
        else:
            unit_v(a, b2)
            done_v.add((a, b2))

    def require(kind, a, b2):
        tgt = (kind, a, b2)
        if kind == "q" and a in done_q:
            return
        if kind == "k" and (a, b2) in done_k:
            return
        if kind == "v" and (a, b2) in done_v:
            return
        assert tgt in units, f"missing unit {tgt}"
        while True:
            nxt = units[0]
            emit_unit()
            if nxt == tgt:
                break

    total_units = 4 + len(units)
    total_steps = NH * NSTEP
    emitted = [4]

    # ---------------- attention (units interleaved, ctx one step behind) ----
    rpool = ctx.enter_context(tc.tile_pool(name="rpool", bufs=3))

    def drain_head(h, cps):
        """Per-head drains.  Normalization stays end-gated: its inputs only
        complete with the last head, so the Tile scheduler cannot hoist the
        broadcast matmuls into the attention stream (hoisted bc matmuls
        head-of-line stall the PE and drop the HAM clock gate: +70us).
        The LAST head's chain alone gates the tail reciprocal, so it gets
        high priority to jump the DVE backlog."""
        hp, hr = h // 2, (h % 2) * HD
        from contextlib import nullcontext
        prio = tc.high_priority() if h == NH - 1 else nullcontext()
        with prio:
            nc.vector.tensor_copy(ctxT_sb[hr : hr + HD, hp, :], cps[0:HD, :])
            stmp = rpool.tile([1, SQ], F32, name="stmp")
            nc.vector.tensor_copy(stmp, cps[HD : HD + 1, :])
            # DMA sidesteps the partition-base restriction on compute engines
            nc.sync.dma_start(out=sums16[h : h + 1, :], in_=stmp)

    pend = [None]  # (h, tg, pT, cps)

    def flush_ctx():
        if pend[0] is None:
            return
        h, tg, pT, cps = pend[0]
        pend[0] = None
        g = h // 8
        for j in range(CG):
            t = tg * CG + j
            nc.tensor.matmul(
                cps[0 : HD + 1, :],
                lhsT=v_g[g][:, t, h % 8, :],
                rhs=pT[:, j, :],
                start=(t == 0), stop=(t == KC - 1),
            )
        if tg == NSTEP - 1:
            drain_head(h, cps)

    for h in range(NH):
        hp, hr, g = h // 2, (h % 2) * HD, h // 8
        require("q", hp, 0)
        ctx_ps = ps_c.tile([P, SQ], F32, name="ctx_ps")
        for tg in range(NSTEP):
            require("k", hp, tg * CG // 4)
            s_ps = ps_s.tile([P, CG, SQ], F32, name="s_ps", tag="s")
            for j in range(CG):
                t = tg * CG + j
                nc.tensor.matmul(
                    s_ps[:, j, :],
                    lhsT=kT_hp[hp][hr : hr + HD, t * P : (t + 1) * P],
                    rhs=qT_hp[hp][hr : hr + HD, :],
                    start=True, stop=True,
                )
            # spreading quota: keep residual units flowing between steps
            step_no = h * NSTEP + tg + 1
            quota = 4 + (total_units - 4) * step_no // total_steps
            while units and emitted[0] < quota:
                emit_unit()
                emitted[0] += 1
            emitted[0] = 4 + (total_units - 4) - len(units)
            flush_ctx()
            eT = work.tile([P, CG, SQ], BF16, name="eT")
            nc.scalar.activation(eT, s_ps, mybir.ActivationFunctionType.Exp,
                                 scale=SCALE)
            pT = work.tile([P, CG, SQ], BF16, name="pT")
            nc.vector.tensor_mul(pT, eT, maskT_sb[:, tg * CG : (tg + 1) * CG, :])
            for j in range(CG):
                require("v", g, tg * CG + j)
            pend[0] = (h, tg, pT, ctx_ps)
    flush_ctx()
    assert not units

    # batched normalization: one reciprocal over all heads, then per-pair
    # PE broadcast (selector matmul stacks both heads' recips) + one mul.
    # ~5x faster than reciprocal(); 18 correct bits >> the bf16 cast below
    nc.vector.reciprocal_approx_fast(rec_f32, sums16)
    nc.vector.tensor_copy(rec_all, rec_f32)
    for m in range(FC // 2):
        bc2 = ps_s.tile([P, CG, SQ], F32, name="s_ps", tag="s")
        for j in range(2):
            nc.tensor.matmul(bc2[:, j, :], lhsT=sel_sb[:, 2 * m + j, :],
                             rhs=rec_all, start=True, stop=True)
        nc.vector.tensor_mul(ctxT_f8[:, 2 * m : 2 * m + 2, :],
                             ctxT_sb[:, 2 * m : 2 * m + 2, :], bc2)

    # PE keep-alive through the reciprocal + normalization-multiply window:
    # gated on the LAST head's ctxT drain (ready exactly when that window
    # opens), never read, lower priority than bc/dense, so they fill the PE
    # idle gap that otherwise drops the HAM clock gate before dense.
    for i in range(16):
        heat = ps_c.tile([P, SQ], F32, name="ctx_ps")
        nc.tensor.matmul(heat, lhsT=ctxT_sb[HD:P, NHP - 1, 0:P],
                         rhs=ctxT_sb[HD:P, NHP - 1, :],
                         start=True, stop=True)

    # ---------------- phase 3: dense + residual + LayerNorm ----------------
    # Dense accumulates in PSUM ([P, 1024] = one ps_s-shaped tile); the
    # residual (x + bd, x64, bf16) is added via an identity-matmul
    # accumulate, so no separate DVE add/drain pass is needed: LayerNorm
    # reads straight from PSUM.  pre is 64x the reference pre-LN tensor;
    # LN is scale-invariant with eps scaled by 64^2.
    ln_pool = ctx.enter_context(tc.tile_pool(name="ln", bufs=2))
    gb_pool = ctx.enter_context(tc.tile_pool(name="gb", bufs=1))
    eps_t = gb_pool.tile([P, 1], F32)
    nc.vector.memset(eps_t, EPS)
    if affine:
        gamma_bc = gb_pool.tile([P, H], F32)
        beta_bc = gb_pool.tile([P, H], F32)
        nc.sync.dma_start(out=gamma_bc, in_=_bcast_ap(d_gamma, P))
        nc.sync.dma_start(out=beta_bc, in_=_bcast_ap(d_beta, P))

    for r in range(SQ // P):
        dps = ps_s.tile([P, 2, 512], F32, name="s_ps", tag="s")
        for nh2 in range(H // 512):
            nc.tensor.matmul(dps[:, nh2, :], lhsT=eye_sb,
                             rhs=xres_sb[:, r, nh2 * 512 : (nh2 + 1) * 512],
                             start=True, stop=False, skip_group_check=True)
            for c2 in range(FC // 2):
                nc.tensor.matmul(
                    dps[:, nh2, :],
                    lhsT=ctxT_f8[:, 2 * c2 : 2 * c2 + 2, r * P : (r + 1) * P],
                    rhs=wd_sb[:, 2 * c2 : 2 * c2 + 2, nh2 * 512 : (nh2 + 1) * 512],
                    start=False, stop=(c2 == FC // 2 - 1),
                    perf_mode=DR, skip_group_check=True,
                )

        stats = ln_pool.tile([P, 2, 6], F32, name="stats")
        nc.vector.bn_stats(stats[:, 0, :], dps[:, 0, :])
        nc.vector.bn_stats(stats[:, 1, :], dps[:, 1, :])
        mv = ln_pool.tile([P, 2], F32, name="mv")
        nc.vector.bn_aggr(mv, stats)
        std = ln_pool.tile([P, 1], F32, name="std")
        nc.scalar.activation(std, mv[:, 1:2], mybir.ActivationFunctionType.Sqrt,
                             bias=eps_t)
        rstd = ln_pool.tile([P, 1], F32, name="rstd")
        nc.vector.reciprocal(rstd, std)
        outv = ln_pool.tile([P, H], F32, name="outv")
        nc.vector.tensor_scalar(outv, dps.rearrange("p c n -> p (c n)"),
                                mv[:, 0:1], rstd,
                                mybir.AluOpType.subtract, mybir.AluOpType.mult)
        if affine:
            nc.vector.tensor_mul(outv, outv, gamma_bc)
            nc.vector.tensor_add(outv, outv, beta_bc)
        nc.sync.dma_start(out=d_out[r], in_=outv)


_nc_cache = {}


def _get_nc(affine):
    if affine not in _nc_cache:
        _nc_cache[affine] = _build_program(affine)
    return _nc_cache[affine]


def kernel(hidden_states, mask, Wq, Wk, Wv, Wd, bd, gamma, beta):
    global last_results
    hidden_states = np.asarray(hidden_states, dtype=np.float32)
    mask = np.asarray(mask)
    Wq = np.asarray(Wq, dtype=np.float32)
    Wk = np.asarray(Wk, dtype=np.float32)
    Wv = np.asarray(Wv, dtype=np.float32)
    Wd = np.asarray(Wd, dtype=np.float32)
    bd = np.asarray(bd, dtype=np.float32)
    gamma = np.asarray(gamma, dtype=np.float32)
    beta = np.asarray(beta, dtype=np.float32)

    affine = bool(np.any(gamma != 1.0) or np.any(beta != 0.0))
    nc = _get_nc(affine)

    sel_np = np.zeros((NH, FC, P), dtype=np.float32)
    for cc in range(FC):
        sel_np[2 * cc, cc, 0:HD] = 1.0
        sel_np[2 * cc + 1, cc, HD:P] = 1.0
    sel_np = sel_np.astype(NP_BF16)
    eye_np = np.eye(P, dtype=np.float32).astype(np.float16)

    # weight packing (scaled x64 into fp8 normal range)
    # wq/wk strips: [hp][p][c*128+n] = W.T[c*128+p, hp*128+n]
    wqT = (np.ascontiguousarray(Wq.T) * WS).astype(NP_FP8).reshape(FC, P, FC, P)
    wqT = np.ascontiguousarray(wqT.transpose(2, 1, 0, 3)).reshape(NHP, P, FC * P)
    wkT = (np.ascontiguousarray(Wk.T) * WS).astype(NP_FP8).reshape(FC, P, FC, P)
    wkT = np.ascontiguousarray(wkT.transpose(2, 1, 0, 3)).reshape(NHP, P, FC * P)
    # wv halves: [g][p][c*512+n] = Wv.T[c*128+p, g*512+n]
    wvT = (np.ascontiguousarray(Wv.T) * WS).astype(NP_FP8).reshape(FC, P, 2, 512)
    wvT = np.ascontiguousarray(wvT.transpose(2, 1, 0, 3)).reshape(2, P, FC * 512)
    wdT = (np.ascontiguousarray(Wd.T) * WS).astype(NP_FP8).reshape(FC, P, H)

    in_maps = []
    for c in range(NCORES):
        b, qi = c // 4, c % 4
        qs = qi * SQ
        # roll the kv axis so this core's own query rows are columns 0..SQ
        xT = np.roll(hidden_states[b].T, -qs, axis=1)
        xT = np.ascontiguousarray(xT).astype(NP_FP8).reshape(FC, P, S)
        maskT = np.roll(mask[b].T, -qs, axis=0)[:, qs : qs + SQ]
        maskT = np.ascontiguousarray(maskT).astype(NP_BF16).reshape(KC, P, SQ)
        xres = ((hidden_states[b, qs : qs + SQ] + bd[None, :]) * WS).astype(
            np.float16)
        in_maps.append({
            "xT": xT,
            "wqT": wqT,
            "wkT": wkT,
            "wvT": wvT,
            "wdT": wdT,
            "maskT": maskT,
            "xres": np.ascontiguousarray(xres.reshape(SQ // P, P, H)),
            "gamma": gamma,
            "beta": beta,
            "sel": sel_np,
            "eye": eye_np,
        })

    trace = os.environ.get("BASS_KERNEL_TRACE", "0") == "1"
    res = run_bass_kernel_spmd(
        nc, in_maps, core_ids=list(range(NCORES)), trace=trace
    )
    last_results = res

    out = np.empty((B, S, H), dtype=np.float32)
    for c in range(NCORES):
        b, qi = c // 4, c % 4
        out[b, qi * SQ : (qi + 1) * SQ] = res.results[c]["out"].reshape(SQ, H)
    return out


# revision 57
# speedup vs baseline: 1.1870x; 1.1870x over previous
"""
Multi-head attention + residual + LayerNorm Trainium2 kernel (8 NeuronCores).

Problem (hardcoded shapes):
    hidden_states [2, 2048, 1024] f32, mask [2, 2048, 2048] int32,
    Wq/Wk/Wv/Wd [1024, 1024] f32, bd/gamma/beta [1024] f32.
    out = LayerNorm(ctx @ Wd.T + bd + hidden_states) with 16 heads, hd=64.

Sharding: pure data parallel. Core c handles batch b = c//4 and query rows
q in [ (c%4)*512, (c%4)*512+512 ).  Each core computes K/V for the full
sequence of its batch (4x redundant), attention + dense + LN for its own
512 rows.  No collectives.

Key engine-balance ideas vs the naive schedule (366us -> 274us):
  * Projections (Q/K/V) and dense run as fp8e4m3 DoubleRow matmuls (2 fp8
    weights per PE cell, ~1.44x measured).  Weights are scaled x64 on host
    so their ~0.02 magnitudes sit in fp8's normal range; the scale is
    folded into the softmax exp scale (q,k), the V ones-row value (v) and
    a scale-invariant LayerNorm with eps*64^2 (dense+residual).
  * Projection units are emitted interleaved into the attention stream on
    a need-by-step basis plus an even spreading quota, so the PE runs at
    98-100% occupancy through the body.  This matters beyond throughput:
    the HAM clock gate halves the PE clock (k=4/8) after PE idle dips, so
    ANY mid-stream instruction that waits on a cross-engine chain (e.g. a
    normalization matmul gated on a DVE reciprocal, or a DVE multiply
    gated on a DMA round-trip) costs ~60-70us via clock flapping.  Hence
    softmax-sum normalization is strictly end-gated: its inputs complete
    only with the last head, so the Tile scheduler cannot hoist it.
  * Dense accumulates in PSUM with the (x64, fp16) residual added via an
    identity-matmul accumulate; LayerNorm (bn_stats / tensor_scalar)
    reads straight from PSUM, so the tail needs no DVE drain pass.
  * Startup: x arrives as fp8 chunk-contiguous (2KB lines) over the
    sync/scalar HW DGE queues plus the gpsimd SW queue; weight strips
    arrive per-head-pair so the first matmul fires after ~1MB of traffic.
"""

import os
import sys
from collections import deque
from contextlib import ExitStack

import numpy as np

for _p in ("/opt/trn_rl_repo",):
    if os.path.isdir(_p) and _p not in sys.path:
        sys.path.insert(0, _p)

import ml_dtypes  # noqa: E402

import concourse.bass as bass  # noqa: E402
import concourse.tile as tile  # noqa: E402
from concourse import bacc, mybir  # noqa: E402
from concourse.bass_utils import run_bass_kernel_spmd  # noqa: E402

BF16 = mybir.dt.bfloat16
F32 = mybir.dt.float32
FP8 = mybir.dt.float8e4
NP_BF16 = ml_dtypes.bfloat16
NP_FP8 = ml_dtypes.float8_e4m3

B, S, H, NH = 2, 2048, 1024, 16
HD = H // NH  # 64
P = 128
NCORES = 8
SQ = S // 4  # 512 query rows per core
FC = H // P  # 8 feature chunks
KC = S // P  # 16 kv chunks
NHP = FC  # 8 head pairs
WS = 64.0  # host-side weight scale (power of 2, keeps fp8 normal)
SCALE = 1.0 / (float(np.sqrt(HD)) * WS * WS)  # exp scale absorbs q,k scales
EPS = 1e-6 * WS * WS  # pre-LN tensor is x64 -> var x64^2; LN is scale-inv
CG = 2  # kv chunks per attention step
NSTEP = KC // CG  # 8 steps per head

DR = mybir.MatmulPerfMode.DoubleRow

# Results of the last device run (for test harness introspection)
last_results = None


def _build_program(affine=True):
    nc = bacc.Bacc(
        "TRN2",
        target_bir_lowering=False,
        debug=False,
        enable_asserts=False,
        num_devices=NCORES,
    )

    # Per-core DRAM inputs.  Weights are fp8, pre-scaled by WS, packed for
    # strip-wise (per head-pair / per v-half) DMA with fat lines.
    d_xT = nc.dram_tensor("xT", [FC, P, S], FP8, kind="ExternalInput").ap()
    d_wq = nc.dram_tensor("wqT", [NHP, P, FC * P], FP8, kind="ExternalInput").ap()
    d_wk = nc.dram_tensor("wkT", [NHP, P, FC * P], FP8, kind="ExternalInput").ap()
    d_wv = nc.dram_tensor("wvT", [2, P, FC * 512], FP8, kind="ExternalInput").ap()
    d_wd = nc.dram_tensor("wdT", [FC, P, H], FP8, kind="ExternalInput").ap()
    d_maskT = nc.dram_tensor("maskT", [KC, P, SQ], BF16, kind="ExternalInput").ap()
    d_xres = nc.dram_tensor("xres", [SQ // P, P, H], mybir.dt.float16,
                            kind="ExternalInput").ap()
    d_gamma = nc.dram_tensor("gamma", [H], F32, kind="ExternalInput").ap()
    d_beta = nc.dram_tensor("beta", [H], F32, kind="ExternalInput").ap()
    d_sel = nc.dram_tensor("sel", [NH, FC, P], BF16, kind="ExternalInput").ap()
    d_eye = nc.dram_tensor("eye", [P, P], mybir.dt.float16,
                           kind="ExternalInput").ap()
    d_out = nc.dram_tensor("out", [SQ // P, P, H], F32, kind="ExternalOutput").ap()

    with tile.TileContext(nc, trace_sim=False) as tc:
        _program(tc, d_xT, d_wq, d_wk, d_wv, d_wd, d_maskT, d_xres, d_gamma,
                 d_beta, d_sel, d_eye, d_out, affine)

    nc.compile()
    return nc


def _bcast_ap(src_1d, parts):
    """AP that replicates a [n] DRAM vector across `parts` partitions."""
    return bass.AP(
        tensor=src_1d.tensor,
        offset=src_1d.offset,
        ap=[[0, parts]] + list(src_1d.ap),
    )


def _program(ctx_or_tc, *args):
    with ExitStack() as ctx:
        _program_inner(ctx, ctx_or_tc, *args)


def _program_inner(ctx, tc, d_xT, d_wq, d_wk, d_wv, d_wd, d_maskT, d_xres,
                   d_gamma, d_beta, d_sel, d_eye, d_out, affine):
    nc = tc.nc

    # ---------------- pools ----------------
    persist = ctx.enter_context(tc.tile_pool(name="persist", bufs=1))
    ps_mm = ctx.enter_context(tc.tile_pool(name="ps_mm", bufs=2, space="PSUM"))
    ps_s = ctx.enter_context(tc.tile_pool(name="ps_s", bufs=2, space="PSUM"))
    ps_c = ctx.enter_context(tc.tile_pool(name="ps_c", bufs=2, space="PSUM"))

    # ---------------- persistent tiles ----------------
    xT_sb = persist.tile([P, FC, S], FP8, name="xT_sb")
    wq_sb = persist.tile([P, NHP, FC, P], FP8, name="wq_sb")
    wk_sb = persist.tile([P, NHP, FC, P], FP8, name="wk_sb")
    wv_sb = persist.tile([P, 2, FC, 512], FP8, name="wv_sb")
    wd_sb = persist.tile([P, FC, H], FP8, name="wd_sb")
    kT_hp = [persist.tile([P, S], BF16, name=f"kT{hp}") for hp in range(NHP)]
    qT_hp = [persist.tile([P, SQ], BF16, name=f"qT{hp}") for hp in range(NHP)]
    v_g = [persist.tile([P, KC, 8, HD + 1], BF16, name=f"v{g}") for g in range(2)]
    ctxT_sb = persist.tile([P, FC, SQ], BF16, name="ctxT_sb")
    ctxT_f8 = persist.tile([P, FC, SQ], FP8, name="ctxT_f8")
    maskT_sb = persist.tile([P, KC, SQ], BF16, name="maskT_sb")
    sums16 = persist.tile([NH, SQ], F32, name="sums16")
    rec_all = persist.tile([NH, SQ], BF16, name="rec_all")
    sel_sb = persist.tile([NH, FC, P], BF16, name="sel_sb")
    eye_sb = persist.tile([P, P], mybir.dt.float16, name="eye_sb")
    xres_sb = persist.tile([P, SQ // P, H], mybir.dt.float16, name="xres_sb")

    # ---------------- DMA prefix (critical-path ordered) ----------------
    # 4 HW DGE queues: sync, scalar, vector, tensor; SW queue: gpsimd.
    # First-needed: x chunks (all 8) + wq strip0 + wk strip0 + wv half0 +
    # mask chunks 0..1.
    for g in range(2):
        nc.gpsimd.memset(v_g[g][:, :, :, HD : HD + 1], WS)
    nc.scalar.dma_start(out=eye_sb, in_=d_eye)
    nc.scalar.dma_start(out=wq_sb[:, 0], in_=d_wq[0].rearrange("p (c n) -> p c n", n=P))
    nc.sync.dma_start(out=wk_sb[:, 0], in_=d_wk[0].rearrange("p (c n) -> p c n", n=P))
    qs = [nc.sync, nc.scalar, nc.sync, nc.scalar, nc.sync, nc.scalar,
          nc.gpsimd, nc.gpsimd]
    for c in range(FC):
        qs[c].dma_start(out=xT_sb[:, c, :], in_=d_xT[c])
    nc.scalar.dma_start(out=wv_sb[:, 0], in_=d_wv[0].rearrange("p (c n) -> p c n", n=512))
    nc.scalar.dma_start(out=maskT_sb[:, 0:4, :],
                        in_=d_maskT[0:4].rearrange("c p n -> p c n"))
    # second wave (needed within the first two head-pairs)
    nc.scalar.dma_start(out=wq_sb[:, 1], in_=d_wq[1].rearrange("p (c n) -> p c n", n=P))
    nc.sync.dma_start(out=wk_sb[:, 1], in_=d_wk[1].rearrange("p (c n) -> p c n", n=P))
    nc.sync.dma_start(out=maskT_sb[:, 4:10, :],
                      in_=d_maskT[4:10].rearrange("c p n -> p c n"))
    nc.scalar.dma_start(out=maskT_sb[:, 10:16, :],
                        in_=d_maskT[10:16].rearrange("c p n -> p c n"))
    nc.sync.dma_start(out=wv_sb[:, 1], in_=d_wv[1].rearrange("p (c n) -> p c n", n=512))
    for hp in range(2, NHP):
        nc.scalar.dma_start(out=wq_sb[:, hp],
                            in_=d_wq[hp].rearrange("p (c n) -> p c n", n=P))
        nc.sync.dma_start(out=wk_sb[:, hp],
                          in_=d_wk[hp].rearrange("p (c n) -> p c n", n=P))
    nc.sync.dma_start(out=sel_sb, in_=d_sel)
    nc.gpsimd.dma_start(out=xres_sb, in_=d_xres.rearrange("r p n -> p r n"))
    nc.sync.dma_start(out=wd_sb, in_=d_wd.rearrange("c p n -> p c n"))

    # PE p-state warmup on early-arriving data (eye 32KB + wq strip 0):
    # ramps the clock before the first projection unit.
    for i in range(5):
        wps = ps_mm.tile([P, SQ], F32, name="qps", tag="mm")
        nc.tensor.matmul(wps, lhsT=eye_sb,
                         rhs=wq_sb[:, 0, :, :].rearrange("p c n -> p (c n)")[:, 0:SQ],
                         start=True, stop=True)

    work = ctx.enter_context(tc.tile_pool(name="work", bufs=3))

    # ---------------- projection units (fp8 DoubleRow) ----------------
    def unit_q(hp):
        qps = ps_mm.tile([P, SQ], F32, name="qps", tag="mm")
        for c2 in range(FC // 2):
            nc.tensor.matmul(qps,
                             lhsT=wq_sb[:, hp, 2 * c2 : 2 * c2 + 2, :],
                             rhs=xT_sb[:, 2 * c2 : 2 * c2 + 2, 0:SQ],
                             start=(c2 == 0), stop=(c2 == FC // 2 - 1),
                             perf_mode=DR)
        nc.vector.tensor_copy(qT_hp[hp], qps)

    def unit_k(hp, n):
        kps = ps_mm.tile([P, 512], F32, name="kps", tag="mm")
        for c2 in range(FC // 2):
            nc.tensor.matmul(kps,
                             lhsT=wk_sb[:, hp, 2 * c2 : 2 * c2 + 2, :],
                             rhs=xT_sb[:, 2 * c2 : 2 * c2 + 2, n * 512 : (n + 1) * 512],
                             start=(c2 == 0), stop=(c2 == FC // 2 - 1),
                             perf_mode=DR)
        nc.vector.tensor_copy(kT_hp[hp][:, n * 512 : (n + 1) * 512], kps)

    def unit_v(g, t):
        vps = ps_mm.tile([P, 512], F32, name="vps", tag="mm")
        for c2 in range(FC // 2):
            nc.tensor.matmul(vps,
                             lhsT=xT_sb[:, 2 * c2 : 2 * c2 + 2, t * P : (t + 1) * P],
                             rhs=wv_sb[:, g, 2 * c2 : 2 * c2 + 2, :],
                             start=(c2 == 0), stop=(c2 == FC // 2 - 1),
                             perf_mode=DR)
        nc.vector.tensor_copy(v_g[g][:, t, :, 0:HD],
                              vps.rearrange("p (h d) -> p h d", d=HD))

    # ---------------- unit schedule ----------------
    # Prefix: minimum to unblock head 0 step 0.
    unit_q(0)
    unit_k(0, 0)
    unit_v(0, 0)
    unit_v(0, 1)

    done_q = {0}
    done_k = {(0, 0)}
    done_v = {(0, 0), (0, 1)}

    # Remaining units ordered by earliest need-by (head, step); v-group 1
    # spread through the hp1..3 region, k strips just-in-time.
    units = deque()
    for t in range(2, KC):  # v(0,*) needed across head 0's steps
        units.append(("v", 0, t))
        if t % 4 == 3 and t // 4 < 4 and (0, t // 4) not in done_k:
            units.append(("k", 0, t // 4))
    for n in range(1, 4):
        if ("k", 0, n) not in units:
            units.append(("k", 0, n))
    for hp in range(1, NHP):
        units.append(("q", hp, 0))
        for n in range(4):
            units.append(("k", hp, n))
        if hp in (1, 2, 3):  # v group 1: 16 units over 3 head pairs
            base = (hp - 1) * 5
            for t in range(base, min(base + 5, KC)):
                units.append(("v", 1, t))
    units.append(("v", 1, 15))

    def emit_unit():
        kind, a, b2 = units.popleft()
        if kind == "q":
            unit_q(a)
            done_q.add(a)
        elif kind == "k":
            unit_k(a, b2)
            done_k.add((a, b2))
        else:
            unit_v(a, b2)
            done_v.add((a, b2))

    def require(kind, a, b2):
        tgt = (kind, a, b2)
        if kind == "q" and a in done_q:
            return
        if kind == "k" and (a, b2) in done_k:
            return
        if kind == "v" and (a, b2) in done_v:
            return
        assert tgt in units, f"missing unit {tgt}"
        while True:
            nxt = units[0]
            emit_unit()
            if nxt == tgt:
                break

    total_units = 4 + len(units)
    total_steps = NH * NSTEP
    emitted = [4]

    # ---------------- attention (units interleaved, ctx one step behind) ----
    rpool = ctx.enter_context(tc.tile_pool(name="rpool", bufs=3))

    def drain_head(h, cps):
        """Per-head drains.  Normalization stays end-gated: its inputs only
        complete with the last head, so the Tile scheduler cannot hoist the
        broadcast matmuls into the attention stream (hoisted bc matmuls
        head-of-line stall the PE and drop the HAM clock gate: +70us).
        The LAST head's chain alone gates the tail reciprocal, so it gets
        high priority to jump the DVE backlog."""
        hp, hr = h // 2, (h % 2) * HD
        from contextlib import nullcontext
        prio = tc.high_priority() if h == NH - 1 else nullcontext()
        with prio:
            nc.vector.tensor_copy(ctxT_sb[hr : hr + HD, hp, :], cps[0:HD, :])
            stmp = rpool.tile([1, SQ], F32, name="stmp")
            nc.vector.tensor_copy(stmp, cps[HD : HD + 1, :])
            # DMA sidesteps the partition-base restriction on compute engines
            nc.sync.dma_start(out=sums16[h : h + 1, :], in_=stmp)

    pend = [None]  # (h, tg, pT, cps)

    def flush_ctx():
        if pend[0] is None:
            return
        h, tg, pT, cps = pend[0]
        pend[0] = None
        g = h // 8
        for j in range(CG):
            t = tg * CG + j
            nc.tensor.matmul(
                cps[0 : HD + 1, :],
                lhsT=v_g[g][:, t, h % 8, :],
                rhs=pT[:, j, :],
                start=(t == 0), stop=(t == KC - 1),
            )
        if tg == NSTEP - 1:
            drain_head(h, cps)

    for h in range(NH):
        hp, hr, g = h // 2, (h % 2) * HD, h // 8
        require("q", hp, 0)
        ctx_ps = ps_c.tile([P, SQ], F32, name="ctx_ps")
        for tg in range(NSTEP):
            require("k", hp, tg * CG // 4)
            s_ps = ps_s.tile([P, CG, SQ], F32, name="s_ps", tag="s")
            for j in range(CG):
                t = tg * CG + j
                nc.tensor.matmul(
                    s_ps[:, j, :],
                    lhsT=kT_hp[hp][hr : hr + HD, t * P : (t + 1) * P],
                    rhs=qT_hp[hp][hr : hr + HD, :],
                    start=True, stop=True,
                )
            # spreading quota: keep residual units flowing between steps
            step_no = h * NSTEP + tg + 1
            quota = 4 + (total_units - 4) * step_no // total_steps
            while units and emitted[0] < quota:
                emit_unit()
                emitted[0] += 1
            emitted[0] = 4 + (total_units - 4) - len(units)
            flush_ctx()
            eT = work.tile([P, CG, SQ], BF16, name="eT")
            nc.scalar.activation(eT, s_ps, mybir.ActivationFunctionType.Exp,
                                 scale=SCALE)
            pT = work.tile([P, CG, SQ], BF16, name="pT")
            nc.vector.tensor_mul(pT, eT, maskT_sb[:, tg * CG : (tg + 1) * CG, :])
            for j in range(CG):
                require("v", g, tg * CG + j)
            pend[0] = (h, tg, pT, ctx_ps)
    flush_ctx()
    assert not units

    # batched normalization: one reciprocal over all heads, then per-pair
    # PE broadcast (selector matmul stacks both heads' recips) + one mul.
    with nc.allow_low_precision(reason="bf16 recip: 0.4% norm err ok"):
        nc.vector.reciprocal(rec_all, sums16)
    for cc in range(FC):
        bc_ps = ps_mm.tile([P, SQ], F32, name="bc_ps", tag="mm")
        nc.tensor.matmul(bc_ps, lhsT=sel_sb[:, cc, :], rhs=rec_all,
                         start=True, stop=True)
        nc.vector.tensor_mul(ctxT_f8[:, cc, :], ctxT_sb[:, cc, :], bc_ps)

    # PE keep-alive through the DVE normalization-multiply window: these are
    # gated on rec_all (tail-only), never read, and lower priority than the
    # bc matmuls, so they run exactly in the PE idle gap that otherwise
    # drops the HAM clock gate before the dense phase.
    for i in range(12):
        heat = ps_c.tile([P, SQ], F32, name="ctx_ps")
        nc.tensor.matmul(heat, lhsT=sel_sb[:, i % FC, :], rhs=rec_all,
                         start=True, stop=True)

    # ---------------- phase 3: dense + residual + LayerNorm ----------------
    # Dense accumulates in PSUM ([P, 1024] = one ps_s-shaped tile); the
    # residual (x + bd, x64, bf16) is added via an identity-matmul
    # accumulate, so no separate DVE add/drain pass is needed: LayerNorm
    # reads straight from PSUM.  pre is 64x the reference pre-LN tensor;
    # LN is scale-invariant with eps scaled by 64^2.
    ln_pool = ctx.enter_context(tc.tile_pool(name="ln", bufs=2))
    gb_pool = ctx.enter_context(tc.tile_pool(name="gb", bufs=1))
    eps_t = gb_pool.tile([P, 1], F32)
    nc.vector.memset(eps_t, EPS)
    if affine:
        gamma_bc = gb_pool.tile([P, H], F32)
        beta_bc = gb_pool.tile([P, H], F32)
        nc.sync.dma_start(out=gamma_bc, in_=_bcast_ap(d_gamma, P))
        nc.sync.dma_start(out=beta_bc, in_=_bcast_ap(d_beta, P))

    for r in range(SQ // P):
        dps = ps_s.tile([P, 2, 512], F32, name="s_ps", tag="s")
        for nh2 in range(H // 512):
            nc.tensor.matmul(dps[:, nh2, :], lhsT=eye_sb,
                             rhs=xres_sb[:, r, nh2 * 512 : (nh2 + 1) * 512],
                             start=True, stop=False, skip_group_check=True)
            for c2 in range(FC // 2):
                nc.tensor.matmul(
                    dps[:, nh2, :],
                    lhsT=ctxT_f8[:, 2 * c2 : 2 * c2 + 2, r * P : (r + 1) * P],
                    rhs=wd_sb[:, 2 * c2 : 2 * c2 + 2, nh2 * 512 : (nh2 + 1) * 512],
                    start=False, stop=(c2 == FC // 2 - 1),
                    perf_mode=DR, skip_group_check=True,
                )

        stats = ln_pool.tile([P, 2, 6], F32, name="stats")
        nc.vector.bn_stats(stats[:, 0, :], dps[:, 0, :])
        nc.vector.bn_stats(stats[:, 1, :], dps[:, 1, :])
        mv = ln_pool.tile([P, 2], F32, name="mv")
        nc.vector.bn_aggr(mv, stats)
        std = ln_pool.tile([P, 1], F32, name="std")
        nc.scalar.activation(std, mv[:, 1:2], mybir.ActivationFunctionType.Sqrt,
                             bias=eps_t)
        rstd = ln_pool.tile([P, 1], F32, name="rstd")
        nc.vector.reciprocal(rstd, std)
        outv = ln_pool.tile([P, H], F32, name="outv")
        nc.vector.tensor_scalar(outv, dps.rearrange("p c n -> p (c n)"),
                                mv[:, 0:1], rstd,
                                mybir.AluOpType.subtract, mybir.AluOpType.mult)
        if affine:
            nc.vector.tensor_mul(outv, outv, gamma_bc)
            nc.vector.tensor_add(outv, outv, beta_bc)
        nc.sync.dma_start(out=d_out[r], in_=outv)


_nc_cache = {}


def _get_nc(affine):
    if affine not in _nc_cache:
        _nc_cache[affine] = _build_program(affine)
    return _nc_cache[affine]


def kernel(hidden_states, mask, Wq, Wk, Wv, Wd, bd, gamma, beta):
    global last_results
    hidden_states = np.asarray(hidden_states, dtype=np.float32)
    mask = np.asarray(mask)
    Wq = np.asarray(Wq, dtype=np.float32)
    Wk = np.asarray(Wk, dtype=np.float32)
    Wv = np.asarray(Wv, dtype=np.float32)
    Wd = np.asarray(Wd, dtype=np.float32)
    bd = np.asarray(bd, dtype=np.float32)
    gamma = np.asarray(gamma, dtype=np.float32)
    beta = np.asarray(beta, dtype=np.float32)

    affine = bool(np.any(gamma != 1.0) or np.any(beta != 0.0))
    nc = _get_nc(affine)

    sel_np = np.zeros((NH, FC, P), dtype=np.float32)
    for cc in range(FC):
        sel_np[2 * cc, cc, 0:HD] = 1.0
        sel_np[2 * cc + 1, cc, HD:P] = 1.0
    sel_np = sel_np.astype(NP_BF16)
    eye_np = np.eye(P, dtype=np.float32).astype(np.float16)

    # weight packing (scaled x64 into fp8 normal range)
    # wq/wk strips: [hp][p][c*128+n] = W.T[c*128+p, hp*128+n]
    wqT = (np.ascontiguousarray(Wq.T) * WS).astype(NP_FP8).reshape(FC, P, FC, P)
    wqT = np.ascontiguousarray(wqT.transpose(2, 1, 0, 3)).reshape(NHP, P, FC * P)
    wkT = (np.ascontiguousarray(Wk.T) * WS).astype(NP_FP8).reshape(FC, P, FC, P)
    wkT = np.ascontiguousarray(wkT.transpose(2, 1, 0, 3)).reshape(NHP, P, FC * P)
    # wv halves: [g][p][c*512+n] = Wv.T[c*128+p, g*512+n]
    wvT = (np.ascontiguousarray(Wv.T) * WS).astype(NP_FP8).reshape(FC, P, 2, 512)
    wvT = np.ascontiguousarray(wvT.transpose(2, 1, 0, 3)).reshape(2, P, FC * 512)
    wdT = (np.ascontiguousarray(Wd.T) * WS).astype(NP_FP8).reshape(FC, P, H)

    in_maps = []
    for c in range(NCORES):
        b, qi = c // 4, c % 4
        qs = qi * SQ
        # roll the kv axis so this core's own query rows are columns 0..SQ
        xT = np.roll(hidden_states[b].T, -qs, axis=1)
        xT = np.ascontiguousarray(xT).astype(NP_FP8).reshape(FC, P, S)
        maskT = np.roll(mask[b].T, -qs, axis=0)[:, qs : qs + SQ]
        maskT = np.ascontiguousarray(maskT).astype(NP_BF16).reshape(KC, P, SQ)
        xres = ((hidden_states[b, qs : qs + SQ] + bd[None, :]) * WS).astype(
            np.float16)
        in_maps.append({
            "xT": xT,
            "wqT": wqT,
            "wkT": wkT,
            "wvT": wvT,
            "wdT": wdT,
            "maskT": maskT,
            "xres": np.ascontiguousarray(xres.reshape(SQ // P, P, H)),
            "gamma": gamma,
            "beta": beta,
            "sel": sel_np,
            "eye": eye_np,
        })

    trace = os.environ.get("BASS_KERNEL_TRACE", "0") == "1"
    res = run_bass_kernel_spmd(
        nc, in_maps, core_ids=list(range(NCORES)), trace=trace
    )
    last_results = res

    out = np.empty((B, S, H), dtype=np.float32)
    for c in range(NCORES):
        b, qi = c // 4, c % 4
        out[b, qi * SQ : (qi + 1) * SQ] = res.results[c]["out"].reshape(SQ, H)
    return out


# revision 58
# speedup vs baseline: 1.1905x; 1.0029x over previous
"""
Multi-head attention + residual + LayerNorm Trainium2 kernel (8 NeuronCores).

Problem (hardcoded shapes):
    hidden_states [2, 2048, 1024] f32, mask [2, 2048, 2048] int32,
    Wq/Wk/Wv/Wd [1024, 1024] f32, bd/gamma/beta [1024] f32.
    out = LayerNorm(ctx @ Wd.T + bd + hidden_states) with 16 heads, hd=64.

Sharding: pure data parallel. Core c handles batch b = c//4 and query rows
q in [ (c%4)*512, (c%4)*512+512 ).  Each core computes K/V for the full
sequence of its batch (4x redundant), attention + dense + LN for its own
512 rows.  No collectives.

Key engine-balance ideas vs the naive schedule (366us -> 274us):
  * Projections (Q/K/V) and dense run as fp8e4m3 DoubleRow matmuls (2 fp8
    weights per PE cell, ~1.44x measured).  Weights are scaled x64 on host
    so their ~0.02 magnitudes sit in fp8's normal range; the scale is
    folded into the softmax exp scale (q,k), the V ones-row value (v) and
    a scale-invariant LayerNorm with eps*64^2 (dense+residual).
  * Projection units are emitted interleaved into the attention stream on
    a need-by-step basis plus an even spreading quota, so the PE runs at
    98-100% occupancy through the body.  This matters beyond throughput:
    the HAM clock gate halves the PE clock (k=4/8) after PE idle dips, so
    ANY mid-stream instruction that waits on a cross-engine chain (e.g. a
    normalization matmul gated on a DVE reciprocal, or a DVE multiply
    gated on a DMA round-trip) costs ~60-70us via clock flapping.  Hence
    softmax-sum normalization is strictly end-gated: its inputs complete
    only with the last head, so the Tile scheduler cannot hoist it.
  * Dense accumulates in PSUM with the (x64, fp16) residual added via an
    identity-matmul accumulate; LayerNorm (bn_stats / tensor_scalar)
    reads straight from PSUM, so the tail needs no DVE drain pass.
  * Startup: x arrives as fp8 chunk-contiguous (2KB lines) over the
    sync/scalar HW DGE queues plus the gpsimd SW queue; weight strips
    arrive per-head-pair so the first matmul fires after ~1MB of traffic.
"""

import os
import sys
from collections import deque
from contextlib import ExitStack

import numpy as np

for _p in ("/opt/trn_rl_repo",):
    if os.path.isdir(_p) and _p not in sys.path:
        sys.path.insert(0, _p)

import ml_dtypes  # noqa: E402

import concourse.bass as bass  # noqa: E402
import concourse.tile as tile  # noqa: E402
from concourse import bacc, mybir  # noqa: E402
from concourse.bass_utils import run_bass_kernel_spmd  # noqa: E402

BF16 = mybir.dt.bfloat16
F32 = mybir.dt.float32
FP8 = mybir.dt.float8e4
NP_BF16 = ml_dtypes.bfloat16
NP_FP8 = ml_dtypes.float8_e4m3

B, S, H, NH = 2, 2048, 1024, 16
HD = H // NH  # 64
P = 128
NCORES = 8
SQ = S // 4  # 512 query rows per core
FC = H // P  # 8 feature chunks
KC = S // P  # 16 kv chunks
NHP = FC  # 8 head pairs
WS = 64.0  # host-side weight scale (power of 2, keeps fp8 normal)
SCALE = 1.0 / (float(np.sqrt(HD)) * WS * WS)  # exp scale absorbs q,k scales
EPS = 1e-6 * WS * WS  # pre-LN tensor is x64 -> var x64^2; LN is scale-inv
CG = 2  # kv chunks per attention step
NSTEP = KC // CG  # 8 steps per head

DR = mybir.MatmulPerfMode.DoubleRow

# Results of the last device run (for test harness introspection)
last_results = None


def _build_program(affine=True):
    nc = bacc.Bacc(
        "TRN2",
        target_bir_lowering=False,
        debug=False,
        enable_asserts=False,
        num_devices=NCORES,
    )

    # Per-core DRAM inputs.  Weights are fp8, pre-scaled by WS, packed for
    # strip-wise (per head-pair / per v-half) DMA with fat lines.
    d_xT = nc.dram_tensor("xT", [FC, P, S], FP8, kind="ExternalInput").ap()
    d_wq = nc.dram_tensor("wqT", [NHP, P, FC * P], FP8, kind="ExternalInput").ap()
    d_wk = nc.dram_tensor("wkT", [NHP, P, FC * P], FP8, kind="ExternalInput").ap()
    d_wv = nc.dram_tensor("wvT", [2, P, FC * 512], FP8, kind="ExternalInput").ap()
    d_wd = nc.dram_tensor("wdT", [FC, P, H], FP8, kind="ExternalInput").ap()
    d_maskT = nc.dram_tensor("maskT", [KC, P, SQ], BF16, kind="ExternalInput").ap()
    d_xres = nc.dram_tensor("xres", [SQ // P, P, H], mybir.dt.float16,
                            kind="ExternalInput").ap()
    d_gamma = nc.dram_tensor("gamma", [H], F32, kind="ExternalInput").ap()
    d_beta = nc.dram_tensor("beta", [H], F32, kind="ExternalInput").ap()
    d_sel = nc.dram_tensor("sel", [NH, FC, P], BF16, kind="ExternalInput").ap()
    d_eye = nc.dram_tensor("eye", [P, P], mybir.dt.float16,
                           kind="ExternalInput").ap()
    d_out = nc.dram_tensor("out", [SQ // P, P, H], F32, kind="ExternalOutput").ap()

    with tile.TileContext(nc, trace_sim=False) as tc:
        _program(tc, d_xT, d_wq, d_wk, d_wv, d_wd, d_maskT, d_xres, d_gamma,
                 d_beta, d_sel, d_eye, d_out, affine)

    nc.compile()
    return nc


def _bcast_ap(src_1d, parts):
    """AP that replicates a [n] DRAM vector across `parts` partitions."""
    return bass.AP(
        tensor=src_1d.tensor,
        offset=src_1d.offset,
        ap=[[0, parts]] + list(src_1d.ap),
    )


def _program(ctx_or_tc, *args):
    with ExitStack() as ctx:
        _program_inner(ctx, ctx_or_tc, *args)


def _program_inner(ctx, tc, d_xT, d_wq, d_wk, d_wv, d_wd, d_maskT, d_xres,
                   d_gamma, d_beta, d_sel, d_eye, d_out, affine):
    nc = tc.nc

    # ---------------- pools ----------------
    persist = ctx.enter_context(tc.tile_pool(name="persist", bufs=1))
    ps_mm = ctx.enter_context(tc.tile_pool(name="ps_mm", bufs=2, space="PSUM"))
    ps_s = ctx.enter_context(tc.tile_pool(name="ps_s", bufs=2, space="PSUM"))
    ps_c = ctx.enter_context(tc.tile_pool(name="ps_c", bufs=2, space="PSUM"))

    # ---------------- persistent tiles ----------------
    xT_sb = persist.tile([P, FC, S], FP8, name="xT_sb")
    wq_sb = persist.tile([P, NHP, FC, P], FP8, name="wq_sb")
    wk_sb = persist.tile([P, NHP, FC, P], FP8, name="wk_sb")
    wv_sb = persist.tile([P, 2, FC, 512], FP8, name="wv_sb")
    wd_sb = persist.tile([P, FC, H], FP8, name="wd_sb")
    kT_hp = [persist.tile([P, S], BF16, name=f"kT{hp}") for hp in range(NHP)]
    qT_hp = [persist.tile([P, SQ], BF16, name=f"qT{hp}") for hp in range(NHP)]
    v_g = [persist.tile([P, KC, 8, HD + 1], BF16, name=f"v{g}") for g in range(2)]
    ctxT_sb = persist.tile([P, FC, SQ], BF16, name="ctxT_sb")
    ctxT_f8 = persist.tile([P, FC, SQ], FP8, name="ctxT_f8")
    maskT_sb = persist.tile([P, KC, SQ], BF16, name="maskT_sb")
    sums16 = persist.tile([NH, SQ], F32, name="sums16")
    rec_all = persist.tile([NH, SQ], BF16, name="rec_all")
    rec_f32 = persist.tile([NH, SQ], F32, name="rec_f32")
    sel_sb = persist.tile([NH, FC, P], BF16, name="sel_sb")
    eye_sb = persist.tile([P, P], mybir.dt.float16, name="eye_sb")
    xres_sb = persist.tile([P, SQ // P, H], mybir.dt.float16, name="xres_sb")

    # ---------------- DMA prefix (critical-path ordered) ----------------
    # 4 HW DGE queues: sync, scalar, vector, tensor; SW queue: gpsimd.
    # First-needed: x chunks (all 8) + wq strip0 + wk strip0 + wv half0 +
    # mask chunks 0..1.
    for g in range(2):
        nc.gpsimd.memset(v_g[g][:, :, :, HD : HD + 1], WS)
    nc.scalar.dma_start(out=eye_sb, in_=d_eye)
    nc.scalar.dma_start(out=wq_sb[:, 0], in_=d_wq[0].rearrange("p (c n) -> p c n", n=P))
    nc.sync.dma_start(out=wk_sb[:, 0], in_=d_wk[0].rearrange("p (c n) -> p c n", n=P))
    qs = [nc.sync, nc.scalar, nc.sync, nc.scalar, nc.sync, nc.scalar,
          nc.gpsimd, nc.gpsimd]
    for c in range(FC):
        qs[c].dma_start(out=xT_sb[:, c, :], in_=d_xT[c])
    nc.scalar.dma_start(out=wv_sb[:, 0], in_=d_wv[0].rearrange("p (c n) -> p c n", n=512))
    nc.scalar.dma_start(out=maskT_sb[:, 0:4, :],
                        in_=d_maskT[0:4].rearrange("c p n -> p c n"))
    # second wave (needed within the first two head-pairs)
    nc.scalar.dma_start(out=wq_sb[:, 1], in_=d_wq[1].rearrange("p (c n) -> p c n", n=P))
    nc.sync.dma_start(out=wk_sb[:, 1], in_=d_wk[1].rearrange("p (c n) -> p c n", n=P))
    nc.sync.dma_start(out=maskT_sb[:, 4:10, :],
                      in_=d_maskT[4:10].rearrange("c p n -> p c n"))
    nc.scalar.dma_start(out=maskT_sb[:, 10:16, :],
                        in_=d_maskT[10:16].rearrange("c p n -> p c n"))
    nc.sync.dma_start(out=wv_sb[:, 1], in_=d_wv[1].rearrange("p (c n) -> p c n", n=512))
    for hp in range(2, NHP):
        nc.scalar.dma_start(out=wq_sb[:, hp],
                            in_=d_wq[hp].rearrange("p (c n) -> p c n", n=P))
        nc.sync.dma_start(out=wk_sb[:, hp],
                          in_=d_wk[hp].rearrange("p (c n) -> p c n", n=P))
    nc.sync.dma_start(out=sel_sb, in_=d_sel)
    nc.gpsimd.dma_start(out=xres_sb, in_=d_xres.rearrange("r p n -> p r n"))
    nc.sync.dma_start(out=wd_sb, in_=d_wd.rearrange("c p n -> p c n"))

    # PE p-state warmup on early-arriving data (eye 32KB + wq strip 0):
    # ramps the clock before the first projection unit.
    for i in range(5):
        wps = ps_mm.tile([P, SQ], F32, name="qps", tag="mm")
        nc.tensor.matmul(wps, lhsT=eye_sb,
                         rhs=wq_sb[:, 0, :, :].rearrange("p c n -> p (c n)")[:, 0:SQ],
                         start=True, stop=True)

    work = ctx.enter_context(tc.tile_pool(name="work", bufs=3))

    # ---------------- projection units (fp8 DoubleRow) ----------------
    def unit_q(hp):
        qps = ps_mm.tile([P, SQ], F32, name="qps", tag="mm")
        for c2 in range(FC // 2):
            nc.tensor.matmul(qps,
                             lhsT=wq_sb[:, hp, 2 * c2 : 2 * c2 + 2, :],
                             rhs=xT_sb[:, 2 * c2 : 2 * c2 + 2, 0:SQ],
                             start=(c2 == 0), stop=(c2 == FC // 2 - 1),
                             perf_mode=DR)
        nc.vector.tensor_copy(qT_hp[hp], qps)

    def unit_k(hp, n):
        kps = ps_mm.tile([P, 512], F32, name="kps", tag="mm")
        for c2 in range(FC // 2):
            nc.tensor.matmul(kps,
                             lhsT=wk_sb[:, hp, 2 * c2 : 2 * c2 + 2, :],
                             rhs=xT_sb[:, 2 * c2 : 2 * c2 + 2, n * 512 : (n + 1) * 512],
                             start=(c2 == 0), stop=(c2 == FC // 2 - 1),
                             perf_mode=DR)
        nc.vector.tensor_copy(kT_hp[hp][:, n * 512 : (n + 1) * 512], kps)

    def unit_v(g, t):
        vps = ps_mm.tile([P, 512], F32, name="vps", tag="mm")
        for c2 in range(FC // 2):
            nc.tensor.matmul(vps,
                             lhsT=xT_sb[:, 2 * c2 : 2 * c2 + 2, t * P : (t + 1) * P],
                             rhs=wv_sb[:, g, 2 * c2 : 2 * c2 + 2, :],
                             start=(c2 == 0), stop=(c2 == FC // 2 - 1),
                             perf_mode=DR)
        nc.vector.tensor_copy(v_g[g][:, t, :, 0:HD],
                              vps.rearrange("p (h d) -> p h d", d=HD))

    # ---------------- unit schedule ----------------
    # Prefix: minimum to unblock head 0 step 0.
    unit_q(0)
    unit_k(0, 0)
    unit_v(0, 0)
    unit_v(0, 1)

    done_q = {0}
    done_k = {(0, 0)}
    done_v = {(0, 0), (0, 1)}

    # Remaining units ordered by earliest need-by (head, step); v-group 1
    # spread through the hp1..3 region, k strips just-in-time.
    units = deque()
    for t in range(2, KC):  # v(0,*) needed across head 0's steps
        units.append(("v", 0, t))
        if t % 4 == 3 and t // 4 < 4 and (0, t // 4) not in done_k:
            units.append(("k", 0, t // 4))
    for n in range(1, 4):
        if ("k", 0, n) not in units:
            units.append(("k", 0, n))
    for hp in range(1, NHP):
        units.append(("q", hp, 0))
        for n in range(4):
            units.append(("k", hp, n))
        if hp in (1, 2, 3):  # v group 1: 16 units over 3 head pairs
            base = (hp - 1) * 5
            for t in range(base, min(base + 5, KC)):
                units.append(("v", 1, t))
    units.append(("v", 1, 15))

    def emit_unit():
        kind, a, b2 = units.popleft()
        if kind == "q":
            unit_q(a)
            done_q.add(a)
        elif kind == "k":
            unit_k(a, b2)
            done_k.add((a, b2))
        else:
            unit_v(a, b2)
            done_v.add((a, b2))

    def require(kind, a, b2):
        tgt = (kind, a, b2)
        if kind == "q" and a in done_q:
            return
        if kind == "k" and (a, b2) in done_k:
            return
        if kind == "v" and (a, b2) in done_v:
            return
        assert tgt in units, f"missing unit {tgt}"
        while True:
            nxt = units[0]
            emit_unit()
            if nxt == tgt:
                break

    total_units = 4 + len(units)
    total_steps = NH * NSTEP
    emitted = [4]

    # ---------------- attention (units interleaved, ctx one step behind) ----
    rpool = ctx.enter_context(tc.tile_pool(name="rpool", bufs=3))

    def drain_head(h, cps):
        """Per-head drains.  Normalization stays end-gated: its inputs only
        complete with the last head, so the Tile scheduler cannot hoist the
        broadcast matmuls into the attention stream (hoisted bc matmuls
        head-of-line stall the PE and drop the HAM clock gate: +70us).
        The LAST head's chain alone gates the tail reciprocal, so it gets
        high priority to jump the DVE backlog."""
        hp, hr = h // 2, (h % 2) * HD
        from contextlib import nullcontext
        prio = tc.high_priority() if h == NH - 1 else nullcontext()
        with prio:
            if h == NH - 1:
                # sums chain first: it alone gates the tail reciprocal
                stmp = rpool.tile([1, SQ], F32, name="stmp")
                nc.vector.tensor_copy(stmp, cps[HD : HD + 1, :])
                nc.sync.dma_start(out=sums16[h : h + 1, :], in_=stmp)
                nc.vector.tensor_copy(ctxT_sb[hr : hr + HD, hp, :], cps[0:HD, :])
            else:
                nc.vector.tensor_copy(ctxT_sb[hr : hr + HD, hp, :], cps[0:HD, :])
                stmp = rpool.tile([1, SQ], F32, name="stmp")
                nc.vector.tensor_copy(stmp, cps[HD : HD + 1, :])
                # DMA sidesteps the partition-base restriction on engines
                nc.sync.dma_start(out=sums16[h : h + 1, :], in_=stmp)

    pend = [None]  # (h, tg, pT, cps)

    def flush_ctx():
        if pend[0] is None:
            return
        h, tg, pT, cps = pend[0]
        pend[0] = None
        g = h // 8
        for j in range(CG):
            t = tg * CG + j
            nc.tensor.matmul(
                cps[0 : HD + 1, :],
                lhsT=v_g[g][:, t, h % 8, :],
                rhs=pT[:, j, :],
                start=(t == 0), stop=(t == KC - 1),
            )
        if tg == NSTEP - 1:
            drain_head(h, cps)

    for h in range(NH):
        hp, hr, g = h // 2, (h % 2) * HD, h // 8
        require("q", hp, 0)
        ctx_ps = ps_c.tile([P, SQ], F32, name="ctx_ps")
        for tg in range(NSTEP):
            require("k", hp, tg * CG // 4)
            s_ps = ps_s.tile([P, CG, SQ], F32, name="s_ps", tag="s")
            for j in range(CG):
                t = tg * CG + j
                nc.tensor.matmul(
                    s_ps[:, j, :],
                    lhsT=kT_hp[hp][hr : hr + HD, t * P : (t + 1) * P],
                    rhs=qT_hp[hp][hr : hr + HD, :],
                    start=True, stop=True,
                )
            # spreading quota: keep residual units flowing between steps
            step_no = h * NSTEP + tg + 1
            quota = 4 + (total_units - 4) * step_no // total_steps
            while units and emitted[0] < quota:
                emit_unit()
                emitted[0] += 1
            emitted[0] = 4 + (total_units - 4) - len(units)
            flush_ctx()
            eT = work.tile([P, CG, SQ], BF16, name="eT")
            nc.scalar.activation(eT, s_ps, mybir.ActivationFunctionType.Exp,
                                 scale=SCALE)
            pT = work.tile([P, CG, SQ], BF16, name="pT")
            nc.vector.tensor_mul(pT, eT, maskT_sb[:, tg * CG : (tg + 1) * CG, :])
            for j in range(CG):
                require("v", g, tg * CG + j)
            pend[0] = (h, tg, pT, ctx_ps)
    flush_ctx()
    assert not units

    # batched normalization: one reciprocal over all heads, then per-pair
    # PE broadcast (selector matmul stacks both heads' recips) + one mul.
    # ~5x faster than reciprocal() (which measured 3.34us on the critical
    # tail chain); 18 correct bits >> the bf16 cast below
    nc.vector.reciprocal_approx_fast(rec_f32, sums16)
    nc.vector.tensor_copy(rec_all, rec_f32)
    for cc in range(FC):
        bc_ps = ps_mm.tile([P, SQ], F32, name="bc_ps", tag="mm")
        nc.tensor.matmul(bc_ps, lhsT=sel_sb[:, cc, :], rhs=rec_all,
                         start=True, stop=True)
        nc.vector.tensor_mul(ctxT_f8[:, cc, :], ctxT_sb[:, cc, :], bc_ps)

    # PE keep-alive through the DVE normalization-multiply window: these are
    # gated on rec_all (tail-only), never read, and lower priority than the
    # bc matmuls, so they run exactly in the PE idle gap that otherwise
    # drops the HAM clock gate before the dense phase.
    for i in range(12):
        heat = ps_c.tile([P, SQ], F32, name="ctx_ps")
        nc.tensor.matmul(heat, lhsT=ctxT_sb[HD:P, NHP - 1, 0:P],
                         rhs=ctxT_sb[HD:P, NHP - 1, :],
                         start=True, stop=True)

    # ---------------- phase 3: dense + residual + LayerNorm ----------------
    # Dense accumulates in PSUM ([P, 1024] = one ps_s-shaped tile); the
    # residual (x + bd, x64, bf16) is added via an identity-matmul
    # accumulate, so no separate DVE add/drain pass is needed: LayerNorm
    # reads straight from PSUM.  pre is 64x the reference pre-LN tensor;
    # LN is scale-invariant with eps scaled by 64^2.
    ln_pool = ctx.enter_context(tc.tile_pool(name="ln", bufs=2))
    gb_pool = ctx.enter_context(tc.tile_pool(name="gb", bufs=1))
    eps_t = gb_pool.tile([P, 1], F32)
    nc.vector.memset(eps_t, EPS)
    if affine:
        gamma_bc = gb_pool.tile([P, H], F32)
        beta_bc = gb_pool.tile([P, H], F32)
        nc.sync.dma_start(out=gamma_bc, in_=_bcast_ap(d_gamma, P))
        nc.sync.dma_start(out=beta_bc, in_=_bcast_ap(d_beta, P))

    for r in range(SQ // P):
        dps = ps_s.tile([P, 2, 512], F32, name="s_ps", tag="s")
        for nh2 in range(H // 512):
            nc.tensor.matmul(dps[:, nh2, :], lhsT=eye_sb,
                             rhs=xres_sb[:, r, nh2 * 512 : (nh2 + 1) * 512],
                             start=True, stop=False, skip_group_check=True)
            for c2 in range(FC // 2):
                nc.tensor.matmul(
                    dps[:, nh2, :],
                    lhsT=ctxT_f8[:, 2 * c2 : 2 * c2 + 2, r * P : (r + 1) * P],
                    rhs=wd_sb[:, 2 * c2 : 2 * c2 + 2, nh2 * 512 : (nh2 + 1) * 512],
                    start=False, stop=(c2 == FC // 2 - 1),
                    perf_mode=DR, skip_group_check=True,
                )

        stats = ln_pool.tile([P, 2, 6], F32, name="stats")
        nc.vector.bn_stats(stats[:, 0, :], dps[:, 0, :])
        nc.vector.bn_stats(stats[:, 1, :], dps[:, 1, :])
        mv = ln_pool.tile([P, 2], F32, name="mv")
        nc.vector.bn_aggr(mv, stats)
        std = ln_pool.tile([P, 1], F32, name="std")
        nc.scalar.activation(std, mv[:, 1:2], mybir.ActivationFunctionType.Sqrt,
                             bias=eps_t)
        rstd = ln_pool.tile([P, 1], F32, name="rstd")
        nc.vector.reciprocal(rstd, std)
        outv = ln_pool.tile([P, H], F32, name="outv")
        nc.vector.tensor_scalar(outv, dps.rearrange("p c n -> p (c n)"),
                                mv[:, 0:1], rstd,
                                mybir.AluOpType.subtract, mybir.AluOpType.mult)
        if affine:
            nc.vector.tensor_mul(outv, outv, gamma_bc)
            nc.vector.tensor_add(outv, outv, beta_bc)
        nc.sync.dma_start(out=d_out[r], in_=outv)


_nc_cache = {}


def _get_nc(affine):
    if affine not in _nc_cache:
        _nc_cache[affine] = _build_program(affine)
    return _nc_cache[affine]


def kernel(hidden_states, mask, Wq, Wk, Wv, Wd, bd, gamma, beta):
    global last_results
    hidden_states = np.asarray(hidden_states, dtype=np.float32)
    mask = np.asarray(mask)
    Wq = np.asarray(Wq, dtype=np.float32)
    Wk = np.asarray(Wk, dtype=np.float32)
    Wv = np.asarray(Wv, dtype=np.float32)
    Wd = np.asarray(Wd, dtype=np.float32)
    bd = np.asarray(bd, dtype=np.float32)
    gamma = np.asarray(gamma, dtype=np.float32)
    beta = np.asarray(beta, dtype=np.float32)

    affine = bool(np.any(gamma != 1.0) or np.any(beta != 0.0))
    nc = _get_nc(affine)

    sel_np = np.zeros((NH, FC, P), dtype=np.float32)
    for cc in range(FC):
        sel_np[2 * cc, cc, 0:HD] = 1.0
        sel_np[2 * cc + 1, cc, HD:P] = 1.0
    sel_np = sel_np.astype(NP_BF16)
    eye_np = np.eye(P, dtype=np.float32).astype(np.float16)

    # weight packing (scaled x64 into fp8 normal range)
    # wq/wk strips: [hp][p][c*128+n] = W.T[c*128+p, hp*128+n]
    wqT = (np.ascontiguousarray(Wq.T) * WS).astype(NP_FP8).reshape(FC, P, FC, P)
    wqT = np.ascontiguousarray(wqT.transpose(2, 1, 0, 3)).reshape(NHP, P, FC * P)
    wkT = (np.ascontiguousarray(Wk.T) * WS).astype(NP_FP8).reshape(FC, P, FC, P)
    wkT = np.ascontiguousarray(wkT.transpose(2, 1, 0, 3)).reshape(NHP, P, FC * P)
    # wv halves: [g][p][c*512+n] = Wv.T[c*128+p, g*512+n]
    wvT = (np.ascontiguousarray(Wv.T) * WS).astype(NP_FP8).reshape(FC, P, 2, 512)
    wvT = np.ascontiguousarray(wvT.transpose(2, 1, 0, 3)).reshape(2, P, FC * 512)
    wdT = (np.ascontiguousarray(Wd.T) * WS).astype(NP_FP8).reshape(FC, P, H)

    in_maps = []
    for c in range(NCORES):
        b, qi = c // 4, c % 4
        qs = qi * SQ
        # roll the kv axis so this core's own query rows are columns 0..SQ
        xT = np.roll(hidden_states[b].T, -qs, axis=1)
        xT = np.ascontiguousarray(xT).astype(NP_FP8).reshape(FC, P, S)
        maskT = np.roll(mask[b].T, -qs, axis=0)[:, qs : qs + SQ]
        maskT = np.ascontiguousarray(maskT).astype(NP_BF16).reshape(KC, P, SQ)
        xres = ((hidden_states[b, qs : qs + SQ] + bd[None, :]) * WS).astype(
            np.float16)
        in_maps.append({
            "xT": xT,
            "wqT": wqT,
            "wkT": wkT,
            "wvT": wvT,
            "wdT": wdT,
            "maskT": maskT,
            "xres": np.ascontiguousarray(xres.reshape(SQ // P, P, H)),
            "gamma": gamma,
            "beta": beta,
            "sel": sel_np,
            "eye": eye_np,
        })

    trace = os.environ.get("BASS_KERNEL_TRACE", "0") == "1"
    res = run_bass_kernel_spmd(
        nc, in_maps, core_ids=list(range(NCORES)), trace=trace
    )
    last_results = res

    out = np.empty((B, S, H), dtype=np.float32)
    for c in range(NCORES):
        b, qi = c // 4, c % 4
        out[b, qi * SQ : (qi + 1) * SQ] = res.results[c]["out"].reshape(SQ, H)
    return out
